# revision 64
# baseline (speedup 1.0000x reference)
"""Causal multi-head self-attention on 8 TRN2 NeuronCores.

Sharding: batch (4) x head-group (2) -> 8 cores. Each core computes, for its
batch b and its 8 heads, the attention output projected through its slice of
Wo; the host sums the two partial outputs per batch.

Per-core layout (P = 128 partitions):
  xT   [1024, 2048] bf16  - x[b].T (d_model on partitions)
  wqT/wkT/wvT [1024, 512] bf16 - weight slices, transposed to [in, out]
  woT  [512, 1024] bf16   - Wo[:, g*512:(g+1)*512].T
  QT/KT [512, 2048] bf16  - head-dim on partitions (pair p -> tile p)
  V    16 tiles [128, 583] bf16 - seq on partitions, per-head 65-col groups
                                  (64 V cols + ones col for row sums) plus
                                  63 pad cols for 128-wide FWL stationaries
  scores computed transposed: S.T[k, q] = K @ Q.T, per head-pair via
  row-group packing (head A rows 0-63, head B rows 64-127, concurrent).
  exp on ACT with fused 1/sqrt(dk) scale; causal via window-trimmed matmuls
  plus [128,128] lower-triangle mask muls split across DVE/gpsimd.
  AV: O.T[128, q] += V_aug.T @ E.T accumulated over k-tiles in PSUM
  (row 64 = softmax denominators; rows 65-127 pad garbage). Normalization
  pipelined across the next chunk's j-loop: split-engine sums evac, fast
  vector reciprocal on [2,512], bf16 PE broadcast, DVE multiply.
  Final projection all-bf16, software-pipelined, bf16 output DMA with
  host-side upcast. PE warmup matmuls cover the DMA-queue spin-up.
"""

import itertools

import numpy as np
import ml_dtypes

import concourse.bass as bass
import concourse.tile as tile
from concourse import bacc, mybir
from concourse import bass_utils

F32 = mybir.dt.float32
F32R = mybir.dt.float32r
BF16 = mybir.dt.bfloat16
NPBF16 = ml_dtypes.bfloat16

B, S, D, H, DK = 4, 2048, 1024, 16, 64
HC = 8          # heads per core
NPAIR = 4       # head pairs per core
OC = 512        # output dims per core (= HC * DK)
KT_N = 16       # seq k-tiles of 128
SCH = 4         # seq chunks of 512
SCALE = 1.0 / np.sqrt(np.float32(DK))

_CACHE = {}


def _emit(nc, tc, dram):
    P = 128
    xT_d, wqT_d, wkT_d, wvT_d, woT_d, tri_d, sel_d, out_d = (
        dram["xT"], dram["wqT"], dram["wkT"], dram["wvT"], dram["woT"],
        dram["trimask"], dram["sel2"], dram["out"],
    )

    import contextlib
    ctx = contextlib.ExitStack()
    with ctx:
        # ---------------- persistent SBUF ----------------
        per = ctx.enter_context(tc.tile_pool(name="per", bufs=1))
        trimask = per.tile([P, P], BF16, tag="trimask", name="trimask")
        nc.sync.dma_start(trimask[:], tri_d[:, :])
        sel2 = per.tile([2, P], BF16, tag="sel2", name="sel2")
        nc.sync.dma_start(sel2[:], sel_d[:, :])

        QT = [per.tile([P, S], BF16, tag=f"QT{p}", name=f"QT{p}") for p in range(NPAIR)]
        KT = [per.tile([P, S], BF16, tag=f"KT{p}", name=f"KT{p}") for p in range(NPAIR)]
        # 63 pad columns so each head's V slice can be read as a full
        # 128-column stationary (FWL-eligible) without going out of bounds
        V = [per.tile([P, HC * 65 + 63], BF16, tag=f"V{t}", name=f"V{t}")
             for t in range(KT_N)]
        OT = [per.tile([P, S], BF16, tag=f"OT{p}", name=f"OT{p}") for p in range(NPAIR)]
        woT = [per.tile([P, D], BF16, tag=f"woT{p}", name=f"woT{p}") for p in range(NPAIR)]

        epool = ctx.enter_context(tc.tile_pool(name="epool", bufs=6))
        stg = ctx.enter_context(tc.tile_pool(name="stg", bufs=1))

        with tc.tile_pool(name="proj_in", bufs=1) as proj_in, \
             tc.tile_pool(name="wqk", bufs=4) as wqk_pool, \
             tc.tile_pool(name="ps", bufs=1, space="PSUM") as psp:
            xT = [proj_in.tile([P, S], BF16, tag=f"xT{k}", name=f"xT{k}") for k in range(8)]
            wvT = [proj_in.tile([P, OC], BF16, tag=f"wvT{k}", name=f"wvT{k}") for k in range(8)]

            # PSUM budget (8 banks of [128,512]f32):
            #   pp: 2 x [128,512]  = 2 banks (projection accumulators)
            #   s:  2 x [128,1024] = 4 banks (scores, both heads)
            #   o:  1 x [128,1024] = 2 banks (output accum, both heads)
            def pp_tile():
                return psp.tile([P, OC], F32, tag="pp", bufs=2, name="pp")

            # ----- QK projections + attention, pair by pair.
            # The PE executes its stream in order and the attention j-loop is
            # ACT(exp)-bound, so projection / final-projection matmuls are
            # interleaved as per-MM filler units inside the j-loop.
            # Weight-slice DMAs for pair p are issued a full pair ahead of
            # the matmuls so filler MMs are never blocked on a fresh load.
            def proj_loads(p):
                boxes = []
                for wd in (wqT_d, wkT_d):
                    wsl = wqk_pool.tile([P, 8 * P], BF16, tag="wsl",
                                        name="wsl")
                    nc.sync.dma_start(
                        wsl[:].rearrange("p (k c) -> p k c", c=P),
                        wd[:, p * P:(p + 1) * P].rearrange(
                            "(k p) c -> p k c", p=P),
                    )
                    boxes.append(wsl)
                return boxes

            def proj_units(p, boxes):
                units = []
                for sc in range(SCH):
                    for (wsl, dst) in zip(boxes, (QT[p], KT[p])):
                        ps_box = []
                        for k in range(8):
                            def mm(k=k, sc=sc, ps_box=ps_box, wsl=wsl):
                                if k == 0:
                                    ps_box.append(pp_tile())
                                nc.tensor.matmul(
                                    ps_box[0][:],
                                    wsl[:, k * P:(k + 1) * P],
                                    xT[k][:, sc * 512:(sc + 1) * 512],
                                    start=(k == 0), stop=(k == 7),
                                )
                            units.append(mm)
                        def cp(sc=sc, ps_box=ps_box, dst=dst):
                            nc.vector.tensor_copy(
                                dst[:, sc * 512:(sc + 1) * 512], ps_box[0][:])
                        units.append(cp)
                return iter(units)

            def v_units(st):
                # V[st] = x[st*128:+128, :] @ WvT, seq on partitions
                ps = pp_tile()
                for k in range(8):
                    nc.tensor.matmul(
                        ps[:], xT[k][:, st * P:(st + 1) * P], wvT[k][:],
                        start=(k == 0), stop=(k == 7),
                    )
                v3 = V[st][:, 0:HC * 65].rearrange("p (h d) -> p h d", d=65)
                nc.vector.tensor_copy(
                    v3[:, :, 0:64], ps[:].rearrange("p (h d) -> p h d", d=64)
                )
                nc.gpsimd.memset(v3[:, :, 64:65], 1.0)
                nc.gpsimd.memset(V[st][:, HC * 65:], 0.0)

            def final_units(cc):
                # Per (t, oc) unit: the p4=0..2 matmuls are emitted ahead
                # of the pair-3 matmul + evac, so while the pair-3 OT chunk
                # is still normalizing the PE runs later units' leading
                # matmuls instead of stalling. The last group (cc==3, the
                # kernel tail) borrows the freed score-psum banks for a
                # deeper pipeline and evacuates on the now-idle scalar
                # engine.
                tail_grp = cc == 3
                depth = 4 if tail_grp else 1
                units = []
                pend = []
                uidx = [0]
                for t in range(4 * cc, 4 * cc + 4):
                    for oc in range(2):
                        ps_box = []
                        use_s = tail_grp and uidx[0] % 2 == 1
                        uidx[0] += 1
                        for p4 in range(NPAIR - 1):
                            def mm(p4=p4, ps_box=ps_box, t=t, oc=oc,
                                   use_s=use_s):
                                if p4 == 0:
                                    if use_s:
                                        s = psp.tile([P, 1024], F32,
                                                     tag="s", bufs=2,
                                                     name="s2")
                                        ps_box.append(s[:, 0:OC])
                                    else:
                                        ps_box.append(
                                            psp.tile([P, OC], F32, tag="pp",
                                                     bufs=2, name="pp")[:])
                                nc.tensor.matmul(
                                    ps_box[0],
                                    OT[p4][:, t * P:(t + 1) * P],
                                    woT[p4][:, oc * 512:(oc + 1) * 512],
                                    start=(p4 == 0), stop=False,
                                )
                            units.append(mm)

                        def tail(t=t, oc=oc, ps_box=ps_box, use_s=use_s):
                            nc.tensor.matmul(
                                ps_box[0],
                                OT[NPAIR - 1][:, t * P:(t + 1) * P],
                                woT[NPAIR - 1][:, oc * 512:(oc + 1) * 512],
                                start=False, stop=True,
                            )
                            ostg = stg.tile([P, 512], BF16, tag="ostg", bufs=6,
                                            name="ostg")
                            if tail_grp and use_s:
                                nc.scalar.copy(ostg[:], ps_box[0])
                            else:
                                nc.vector.tensor_copy(ostg[:], ps_box[0])
                            nc.sync.dma_start(
                                out_d[t * P:(t + 1) * P,
                                      oc * 512:(oc + 1) * 512],
                                ostg[:],
                            )
                        if len(pend) >= depth:
                            units.append(pend.pop(0))
                        # drain faster near the group's end so fewer evacs
                        # trail the last matmul
                        if tail_grp and uidx[0] >= 6 and pend:
                            units.append(pend.pop(0))
                        pend.append(tail)
                units.extend(pend)
                return iter(units)

            norm_q = []
            post_q = []
            # ---- PE warmup: the DMA queues take ~10us to start streaming;
            # run dummy matmuls on zeroed scratch so the HAM clock-gate is
            # already at 8/8 when the first real matmul issues (cold MMs
            # run at 1.2 GHz and the warmup window is ~3.4us of activity).
            warm = per.tile([P, 512], BF16, tag="warm", name="warm")
            nc.gpsimd.memset(warm[:], 0.0)
            for _ in range(44):
                ws = psp.tile([P, 1024], F32, tag="s", bufs=2, name="s2")
                nc.tensor.matmul(ws[:, 0:512], warm[:, 0:P], warm[:],
                                 start=True, stop=True)
            # DMA issue order tracks first use: wvT k-interleaved with the
            # leading xT strips (V matmuls start after ~0.6MB), then pair-0
            # weight slices, then the remaining xT quarters.
            for k in range(8):
                nc.sync.dma_start(wvT[k][:], wvT_d[k * P:(k + 1) * P, :])
                nc.sync.dma_start(
                    xT[k][:, 0:256], xT_d[k * P:(k + 1) * P, 0:256])
            for k in range(8):
                nc.sync.dma_start(
                    xT[k][:, 256:512], xT_d[k * P:(k + 1) * P, 256:512])
            boxes0 = proj_loads(0)
            for q in range(1, 4):
                for k in range(8):
                    nc.sync.dma_start(
                        xT[k][:, q * 512:(q + 1) * 512],
                        xT_d[k * P:(k + 1) * P, q * 512:(q + 1) * 512])
            boxes1 = proj_loads(1)
            # V projection and pair-0 QK projection interleaved in xT
            # arrival order so the PE never outruns the HBM feed at start.
            # The last query-chunk of pair-0's projections (needed only by
            # its chunk-3 attention) is left to pair-0's filler stream.
            u0 = proj_units(0, boxes0)
            for q in range(4):
                for st in range(4 * q, 4 * q + 4):
                    v_units(st)
                if q < 3:
                    for _ in range(18):
                        fn = next(u0, None)
                        if fn is not None:
                            fn()
            pu = itertools.chain(u0, proj_units(1, boxes1))
            for p in range(NPAIR):
                if p == 0:
                    boxes2 = proj_loads(2)
                    fillers = [pu, pu, pu, pu]
                    nxt = proj_units(2, boxes2)
                elif p == 1:
                    boxes3 = proj_loads(3)
                    fillers = [pu, pu, pu, pu]
                    nxt = proj_units(3, boxes3)
                elif p == 2:
                    for pq in range(NPAIR):
                        nc.sync.dma_start(
                            woT[pq][:], woT_d[pq * P:(pq + 1) * P, :])
                    fillers = [pu, pu, pu, pu]
                    # leftover pair-3 projection units (late-sc tails) keep
                    # flowing as pair-3 chunk-0 filler
                    nxt = pu
                else:
                    fillers = [pu, final_units(0), final_units(1),
                               final_units(2)]
                    nxt = iter(())
                _attention_pair(nc, tc, psp, epool, stg, p, QT, KT, V, OT,
                                sel2, trimask, norm_q, post_q, fillers, nxt)
                pu = nxt
            for fn in post_q:
                fn()
            for fn in norm_q:
                fn()
            for fn in final_units(3):
                fn()

def _attention_pair(nc, tc, psp, epool, stg, p, QT, KT, V, OT, sel2,
                    trimask, norm_q, post_q, fillers, next_first):
    P = 128
    fillers_ext = list(fillers) + [next_first]

    def fill(it, n):
        for _ in range(n):
            fn = next(it, None)
            if fn is None:
                return
            fn()

    for c in range(SCH):
        filler = fillers[c]
        # final-unit fillers (pair 3, chunks >= 1) must not be emitted
        # before the previous chunk's norm pops at j==3, else the PE
        # waits on a DVE mul that depends on PE work emitted later.
        # Those chunks' early j's are full-width (PE-dense) anyway.
        fill_from = 4 if (p == NPAIR - 1 and c >= 1) else 0
        o2 = psp.tile([P, 1024], F32, tag="o", bufs=1, name="o2")
        njt = 4 * c + 4
        pend_av = []
        for j in range(njt):
            d = j - 4 * c
            w = d * P if d >= 0 else 0
            # the chunk's first scores wait on the previous chunk's
            # second-to-last exp (s rotation); give the in-order PE two
            # filler matmuls ahead of that head-of-line wait
            if j <= 1 and fill_from == 0:
                if j == 0:
                    fill(filler, 5 if c == 0 else 2)
                elif c == 0:
                    fill(filler, 1)
            s2 = psp.tile([P, 1024], F32, tag="s", bufs=2, name="s2")
            for hh in range(2):
                nc.tensor.matmul(
                    s2[:, hh * 512 + w: hh * 512 + 512],
                    KT[p][hh * 64:(hh + 1) * 64, j * P:(j + 1) * P],
                    QT[p][hh * 64:(hh + 1) * 64, c * 512 + w:(c + 1) * 512],
                    start=True, stop=True,
                )
            e2 = epool.tile([P, 1024], BF16, tag="e", name="e2")
            nc.scalar.activation(
                e2[:].rearrange("p (h q) -> p h q", h=2)[:, :, w:512],
                s2[:].rearrange("p (h q) -> p h q", h=2)[:, :, w:512],
                mybir.ActivationFunctionType.Exp,
                scale=float(SCALE),
            )
            if d >= 0:
                # one mask mul per engine; head A (the first AV dep) on
                # the faster vector engine
                blk = e2[:, w: w + P]
                nc.vector.tensor_mul(blk, blk, trimask[:])
                blk = e2[:, 512 + w: 512 + w + P]
                nc.gpsimd.tensor_mul(blk, blk, trimask[:])
            # previous chunk's norm pipeline stages, spread over this
            # chunk's j-loop so their deps are met before they hit a queue
            if j >= 2 and post_q:
                post_q.pop(0)()
            if j == (3 if njt == 4 else 4) and norm_q:
                norm_q.pop(0)()
            # diag j's have narrow matmuls but near-full exp cost, so the
            # PE needs more filler there to stay busy (and warm); rate 2
            # underconsumes slightly so leftovers flow into the starved
            # first chunk of the next pair
            if j >= fill_from:
                fill(filler, (3 if p == NPAIR - 1 else 2) if d >= 0 else 1)
            # first chunk after a boundary: extra AV deferral so o2's
            # previous-chunk evacuation has fully drained
            if len(pend_av) == (3 if c == 0 else 2):
                pend_av.pop(0)()

            def av(j=j, w=w, e2=e2, o2=o2, njt=njt):
                # full 128-column stationary (FWL-eligible LDWEIGHTS):
                # rows 65-127 of the output accumulate garbage from the
                # neighboring head's V columns, in PSUM rows nothing reads
                for hh in range(2):
                    head = 2 * p + hh
                    nc.tensor.matmul(
                        o2[0:128, hh * 512 + w: hh * 512 + 512],
                        V[j][:, head * 65: head * 65 + 128],
                        e2[:, hh * 512 + w: hh * 512 + 512],
                        start=(j == 0), stop=(j == njt - 1),
                    )
            pend_av.append(av)
        for fn in pend_av:
            fn()
        # PSUM -> SBUF staging (engines cannot shift partitions; DMA cannot
        # read PSUM), then SBUF->SBUF DMAs to place head B rows. Sums evac
        # first so o2 frees as soon as possible for the next chunk.
        # sums row evacuates split across scalar (head A, fits the ACT
        # bubble after the chunk's last exp) and vector (head B); stage on
        # DVE. o2 frees after the slowest of the three.
        sums = stg.tile([1, 1024], F32, tag="sums", bufs=2, name="sums")
        nc.scalar.copy(sums[0:1, 0:512], o2[64:65, 0:512])
        nc.vector.tensor_copy(sums[0:1, 512:1024], o2[64:65, 512:1024])
        stage = stg.tile([64, 1024], BF16, tag="stage", bufs=2, name="stage")
        nc.vector.tensor_copy(stage[:], o2[0:64, :])
        nc.sync.dma_start(
            OT[p][0:64, c * 512:(c + 1) * 512], stage[0:64, 0:512])
        nc.sync.dma_start(
            OT[p][64:128, c * 512:(c + 1) * 512], stage[0:64, 512:1024])
        # Hop the two sums rows down to partitions 0-1 (512 elems/lane on
        # 2 lanes beats 1 lane x 1024 for the reciprocal).
        sb2 = stg.tile([2, 512], F32, tag="sb2", bufs=3, name="sb2")
        for hh in range(2):
            nc.sync.dma_start(
                sb2[hh:hh + 1, :], sums[0:1, hh * 512:(hh + 1) * 512])
        rb2 = stg.tile([2, 512], BF16, tag="rb2", bufs=3, name="rb2")

        def _recip(sb2=sb2, rb2=rb2):
            nc.vector.reciprocal_approx_fast(sb2[:], sb2[:])
            nc.vector.tensor_copy(rb2[:], sb2[:])
        post_q.append(_recip)

        def _norm(rb2=rb2, p=p, c=c):
            # Deferred into the next chunk's j-loop (j==3): by then the
            # staged reciprocal chain is complete, so the bc matmul never
            # stalls the in-order PE.
            bc = psp.tile([P, OC], F32, tag="pp", bufs=2, name="pp")
            nc.tensor.matmul(bc[:], sel2[:], rb2[:], start=True, stop=True)
            nc.vector.tensor_mul(
                OT[p][:, c * 512:(c + 1) * 512],
                OT[p][:, c * 512:(c + 1) * 512],
                bc[:],
            )
        norm_q.append(_norm)
        # drain leftover filler only when the next chunk (or the next
        # pair's first chunk) doesn't continue the same iterator
        if fillers_ext[c + 1] is not filler:
            fill(filler, 10 ** 6)


def _build():
    if "nc" in _CACHE:
        return _CACHE["nc"]
    nc = bacc.Bacc("TRN2", target_bir_lowering=False, debug=False)
    dram = {
        "xT": nc.dram_tensor("xT", [D, S], BF16, kind="ExternalInput").ap(),
        "wqT": nc.dram_tensor("wqT", [D, OC], BF16, kind="ExternalInput").ap(),
        "wkT": nc.dram_tensor("wkT", [D, OC], BF16, kind="ExternalInput").ap(),
        "wvT": nc.dram_tensor("wvT", [D, OC], BF16, kind="ExternalInput").ap(),
        "woT": nc.dram_tensor("woT", [OC, D], BF16, kind="ExternalInput").ap(),
        "trimask": nc.dram_tensor("trimask", [128, 128], BF16,
                                  kind="ExternalInput").ap(),
        "sel2": nc.dram_tensor("sel2", [2, 128], BF16,
                               kind="ExternalInput").ap(),
        "out": nc.dram_tensor("out", [S, D], BF16, kind="ExternalOutput").ap(),
    }
    with tile.TileContext(nc) as tc:
        _emit(nc, tc, dram)
    nc.compile()
    _CACHE["nc"] = nc
    return nc


def make_in_maps(x, Wq, Wk, Wv, Wo):
    x = np.asarray(x, np.float32)
    Wq = np.asarray(Wq, np.float32)
    Wk = np.asarray(Wk, np.float32)
    Wv = np.asarray(Wv, np.float32)
    Wo = np.asarray(Wo, np.float32)
    tri = np.tril(np.ones((128, 128), np.float32)).T.astype(NPBF16)
    sel = np.zeros((2, 128), NPBF16)
    sel[0, 0:64] = 1.0
    sel[1, 64:128] = 1.0
    in_maps = []
    for c in range(8):
        b, g = divmod(c, 2)
        sl = slice(g * OC, (g + 1) * OC)
        in_maps.append({
            "xT": np.ascontiguousarray(x[b].T).astype(NPBF16),
            "wqT": np.ascontiguousarray(Wq[sl, :].T).astype(NPBF16),
            "wkT": np.ascontiguousarray(Wk[sl, :].T).astype(NPBF16),
            "wvT": np.ascontiguousarray(Wv[sl, :].T).astype(NPBF16),
            "woT": np.ascontiguousarray(Wo[:, sl].T).astype(NPBF16),
            "trimask": tri,
            "sel2": sel,
        })
    return in_maps


def combine(results):
    parts = [results[c]["out"].astype(np.float32) for c in range(8)]
    return np.stack([parts[2 * b] + parts[2 * b + 1] for b in range(B)])


def kernel(**inputs):
    nc = _build()
    in_maps = make_in_maps(inputs["x"], inputs["Wq"], inputs["Wk"],
                           inputs["Wv"], inputs["Wo"])
    res = bass_utils.run_bass_kernel_spmd(nc, in_maps, core_ids=list(range(8)))
    return combine(res.results)


def run_traced(**inputs):
    nc = _build()
    in_maps = make_in_maps(inputs["x"], inputs["Wq"], inputs["Wk"],
                           inputs["Wv"], inputs["Wo"])
    res = bass_utils.run_bass_kernel_spmd(
        nc, in_maps, core_ids=list(range(8)), trace=True)
    return combine(res.results), res

